# revision 3
# baseline (speedup 1.0000x reference)
"""CLD sde_reverse (Riemann geometry) Trainium2 kernel — v2.

Contract: kernel(u, score_x, t) -> (drift, diffusion), full (unsharded) numpy
arrays, computed on 8 NeuronCores via bass/Tile + run_bass_kernel_spmd.

Sharding: pixels (image rows) are sharded 8 ways; every core sees all 64 batch
elements for its 32 rows. All math is per-pixel 3x3 — no collectives.

v2 design (from HW microbenchmarks):
  - DVE tensor_tensor runs at 1x (0.56 ns/elem) regardless of dtype; the
    baseline was DVE-bound at ~94% busy.  v2 load-balances the big batched
    elementwise work across DVE / GpSimd / ACT / PE:
      * products (coef x data) on DVE, a few on GpSimd (1.93 ns/elem)
      * per-channel sums accumulated on the PE via identity matmuls
        (379 ns / 512-free matmul sustained), drained to fp16 by ACT
      * the dr BG*r term rides the same PSUM accumulation through a
        BG-scaled identity weight tile; the drain applies the -1 sign
      * stage A batch-reduction: PE chunk-accumulation (8 matmuls/term)
        for most terms + short fp16 fold chains on DVE for the rest
      * squares on ACT, crosses on DVE/GpSimd
  - 3x3 chol/inverse stays on small [P,64] planes; ACT Sqrt used directly
    (no Newton step).

Device layout per core: pixel p in [0,8192) maps to (part, pl) = (p>>6, p&63);
tensors are [channel, 128 part, 64 batch, 64 pl] so every DMA run is
contiguous.
"""

import math

import numpy as np

# ---- model constants (from the reference config) ----
M_INV = 4.0
GAMMA_BIG = 0.04
BETA0 = 4.0
RIEMANN_MIX = 0.5
K_DECAY = 4.5
C = 3
HW = 256
B = 64

N_CORES = 8
ROWS = HW // N_CORES  # 32 rows per core
P = 128               # SBUF partitions
PL = (ROWS * HW) // P  # 64 free pixels per partition

BETA_C = BETA0 * math.sqrt(M_INV)        # 8.0
GAMMA_C = GAMMA_BIG * math.sqrt(M_INV)   # 0.08
BG = BETA_C * GAMMA_C                    # 0.64
BG_SCALE = math.sqrt(2.0 * BETA_C * GAMMA_C)

_PROG_CACHE: dict = {}

# ---- engine assignment (tuned against NTFF profiles) ----
GP_CROSS = {(0, 1)}          # cross products on GpSimd
PE_REDUCE = {(0, 0), (1, 1), (2, 2), (0, 1)}   # stage-A terms reduced on PE
GP_DR2 = True                # dr channel-2 products on GpSimd
DX_PE = (True, True, False)  # dx channel sums on PE (else DVE adds)
DR_PE = (False, True, True)  # dr channel assembly on PE (else DVE TS+sub)


def _build_program(ca: float, cid: float, n_reps: int = 1):
    """Build + compile the per-core SPMD bass program.

    ca  = alpha / (B * normalization)   (scale for the raw sum S_ij)
    cid = (1 - alpha) / M_INV           (identity mixture term)
    """
    from contextlib import ExitStack

    import concourse.bacc as bacc
    import concourse.mybir as mybir
    import concourse.tile as tile

    dt = mybir.dt
    op = mybir.AluOpType
    f32 = dt.float32
    f16 = dt.float16
    AF = mybir.ActivationFunctionType

    nc = bacc.Bacc("TRN2", target_bir_lowering=False, debug=False,
                   num_devices=N_CORES)

    s_in = nc.dram_tensor("s_in", [C, P, B, PL], f16,
                          kind="ExternalInput").ap()
    u_in = nc.dram_tensor("u_in", [2 * C, P, B, PL], f16,
                          kind="ExternalInput").ap()
    id_in = nc.dram_tensor("ident", [P, P], dt.float16,
                           kind="ExternalInput").ap()
    drift_o = nc.dram_tensor("drift", [2 * C, P, B, PL], f16,
                             kind="ExternalOutput").ap()
    dif_o = nc.dram_tensor("dif", [C, P, PL], f32, kind="ExternalOutput").ap()

    HB = B // 2   # batch half
    QB = B // 4   # batch quarter

    with tile.TileContext(nc) as tc:
      for _rep in range(n_reps):
        with ExitStack() as stack:
            coef = stack.enter_context(tc.tile_pool(name="coef", bufs=1))
            data = stack.enter_context(tc.tile_pool(name="data", bufs=1))
            tmp = stack.enter_context(tc.tile_pool(name="tmp", bufs=2))
            ident = coef.tile([P, P], f16, tag="ident")
            bgid = coef.tile([P, P], f16, tag="bgid")

            g = {}
            with tc.tile_pool(name="score", bufs=1) as score_pool, \
                 tc.tile_pool(name="prod", bufs=1) as prod_pool, \
                 tc.tile_pool(name="psA", bufs=1, space="PSUM") as psA:
                # ---------------- input DMA ----------------
                s_t = [score_pool.tile([P, B, PL], f16, tag=f"s{c}",
                                       name=f"s{c}") for c in range(C)]
                for bh in range(2):
                    bsl = slice(bh * HB, (bh + 1) * HB)
                    for c in range(C):
                        nc.sync.dma_start(out=s_t[c][:, bsl, :],
                                          in_=s_in[c, :, bsl, :])
                nc.sync.dma_start(out=ident[:], in_=id_in[:])
                nc.scalar.mul(bgid[:], ident[:], BG)
                # u: x channels first (dr path starts earlier), then r
                u_t = []
                for c in range(2 * C):
                    ut = data.tile([P, B, PL], f16, tag=f"u{c}")
                    for bh in range(2):
                        bsl = slice(bh * HB, (bh + 1) * HB)
                        nc.sync.dma_start(out=ut[:, bsl, :],
                                          in_=u_in[c, :, bsl, :])
                    u_t.append(ut)
                x_t, r_t = u_t[:C], u_t[C:]

                # ---------------- stage A ----------------
                # products: squares on ACT, crosses on DVE/GP (b-halved so
                # work starts as soon as the first half-DMAs land)
                q = {}
                for i in range(C):
                    sq = prod_pool.tile([P, B, PL], f16, tag=f"sq{i}")
                    for bh in range(2):
                        bsl = slice(bh * HB, (bh + 1) * HB)
                        nc.scalar.activation(sq[:, bsl, :], s_t[i][:, bsl, :],
                                             AF.Square)
                    q[(i, i)] = sq
                for (i, j) in [(0, 1), (0, 2), (1, 2)]:
                    qt = prod_pool.tile([P, B, PL], f16, tag=f"q{i}{j}")
                    eng = nc.gpsimd if (i, j) in GP_CROSS else nc.vector
                    if (i, j) in GP_CROSS:
                        eng.tensor_tensor(qt[:], s_t[i][:], s_t[j][:],
                                          op.mult)
                    else:
                        for bh in range(2):
                            bsl = slice(bh * HB, (bh + 1) * HB)
                            eng.tensor_tensor(qt[:, bsl, :],
                                              s_t[i][:, bsl, :],
                                              s_t[j][:, bsl, :], op.mult)
                    q[(i, j)] = qt

                # reductions over batch -> S planes [P, PL] f32-ish
                S = {}
                for key in [(0, 0), (1, 1), (2, 2), (0, 1), (0, 2), (1, 2)]:
                    qt = q[key]
                    if key in PE_REDUCE:
                        ps = psA.tile([P, 8 * PL], f32, tag=f"psA{key[0]}{key[1]}")
                        for ck in range(8):
                            rhs = qt[:, ck * 8:(ck + 1) * 8, :].rearrange(
                                "p b l -> p (b l)")
                            nc.tensor.matmul(ps[:], ident[:], rhs,
                                             start=(ck == 0), stop=(ck == 7))
                        r8 = tmp.tile([P, 8, PL], f32, tag="r8")
                        nc.scalar.copy(
                            r8[:], ps[:].rearrange("p (b l) -> p b l", b=8))
                        r4 = tmp.tile([P, 4, PL], f32, tag="r4")
                        nc.vector.tensor_tensor(r4[:], r8[:, 0:4, :],
                                                r8[:, 4:8, :], op.add)
                        r2 = tmp.tile([P, 2, PL], f32, tag="r2")
                        nc.vector.tensor_tensor(r2[:], r4[:, 0:2, :],
                                                r4[:, 2:4, :], op.add)
                        st = tmp.tile([P, PL], f32, tag=f"S{key[0]}{key[1]}")
                        nc.vector.tensor_tensor(st[:], r2[:, 0, :],
                                                r2[:, 1, :], op.add)
                        S[key] = st
                    else:
                        # fp16 fold chain 64 -> 1 on DVE
                        cur = qt
                        w = B
                        while w > 1:
                            h = w // 2
                            nxt = tmp.tile([P, h, PL], f16, tag=f"f{h}",
                                           bufs=3)
                            nc.vector.tensor_tensor(
                                nxt[:], cur[:, 0:h, :], cur[:, h:w, :],
                                op.add)
                            cur = nxt
                            w = h
                        st = tmp.tile([P, PL], f32, tag=f"S{key[0]}{key[1]}")
                        nc.scalar.copy(st[:], cur[:, 0, :])
                        S[key] = st

                for i in range(C):
                    gii = coef.tile([P, PL], f32, tag=f"g{i}{i}")
                    nc.scalar.activation(gii[:], S[(i, i)][:], AF.Copy,
                                         bias=float(cid), scale=float(ca))
                    g[(i, i)] = gii
                for (i, j) in [(0, 1), (0, 2), (1, 2)]:
                    gij = coef.tile([P, PL], f32, tag=f"g{i}{j}")
                    nc.scalar.mul(gij[:], S[(i, j)][:], float(ca))
                    g[(i, j)] = gij
                    g[(j, i)] = gij

            # ------------- stage B: per-pixel 3x3 coefficients -------------
            def tt(a, b_, o, tag):
                t = coef.tile([P, PL], f32, tag=tag)
                nc.vector.tensor_tensor(t[:], a[:], b_[:], o)
                return t

            def fmsub(a, b_, c_, d_, tag):
                # a*b - c*d
                t1 = tmp.tile([P, PL], f32, tag="fm1")
                nc.vector.tensor_tensor(t1[:], a[:], b_[:], op.mult)
                t2 = tmp.tile([P, PL], f32, tag="fm2")
                nc.vector.tensor_tensor(t2[:], c_[:], d_[:], op.mult)
                t = coef.tile([P, PL], f32, tag=tag)
                nc.vector.tensor_tensor(t[:], t1[:], t2[:], op.subtract)
                return t

            def to16(plane, tag):
                e = coef.tile([P, 1, PL], f16, tag=tag)
                nc.scalar.copy(e[:, 0, :], plane[:])
                return e

            def sqrt_p(a, tag):
                out = coef.tile([P, PL], f32, tag=tag)
                nc.scalar.activation(out[:], a[:], AF.Sqrt)
                return out

            l00 = sqrt_p(g[0, 0], "l00")
            il00 = coef.tile([P, PL], f32, tag="il00")
            nc.vector.reciprocal(il00[:], l00[:])
            l10 = tt(g[0, 1], il00, op.mult, "l10")
            l20 = tt(g[0, 2], il00, op.mult, "l20")
            t = tt(l10, l10, op.mult, "l10sq")
            dd1 = tt(g[1, 1], t, op.subtract, "dd1")
            l11 = sqrt_p(dd1, "l11")
            il11 = coef.tile([P, PL], f32, tag="il11")
            nc.vector.reciprocal(il11[:], l11[:])
            t = tt(l20, l10, op.mult, "l20l10")
            t = tt(g[1, 2], t, op.subtract, "g12m")
            l21 = tt(t, il11, op.mult, "l21")
            t = tt(l20, l20, op.mult, "l20sq")
            dd2 = tt(g[2, 2], t, op.subtract, "dd2a")
            t = tt(l21, l21, op.mult, "l21sq")
            dd2 = tt(dd2, t, op.subtract, "dd2")
            l22 = sqrt_p(dd2, "l22")

            # bL = beta * L  (scaled once, reused by drift_r, A, diffusion)
            L = {}
            for (i, j), lt in [((0, 0), l00), ((1, 0), l10), ((1, 1), l11),
                               ((2, 0), l20), ((2, 1), l21), ((2, 2), l22)]:
                blt = coef.tile([P, PL], f32, tag=f"bl{i}{j}")
                nc.scalar.mul(blt[:], lt[:], BETA_C)
                L[(i, j)] = blt
            eL = {(i, j): to16(L[(i, j)], f"eL{i}{j}")[:]
                  for (i, j) in [(0, 0), (1, 0), (1, 1),
                                 (2, 0), (2, 1), (2, 2)]}

            # diffusion_r rows (batch-independent): bg/beta * row sums of bL
            bgob = BG_SCALE / BETA_C
            dif0 = coef.tile([P, PL], f32, tag="dif0")
            nc.scalar.mul(dif0[:], L[0, 0][:], bgob)
            t = tt(L[1, 0], L[1, 1], op.add, "difs1")
            dif1 = coef.tile([P, PL], f32, tag="dif1")
            nc.scalar.mul(dif1[:], t[:], bgob)
            t = tt(L[2, 0], L[2, 1], op.add, "difs2a")
            t = tt(t, L[2, 2], op.add, "difs2")
            dif2 = coef.tile([P, PL], f32, tag="dif2")
            nc.scalar.mul(dif2[:], t[:], bgob)
            for i, dtile in enumerate((dif0, dif1, dif2)):
                nc.sync.dma_start(out=dif_o[i], in_=dtile[:])

            # adjugate (symmetric): c00 = g11*g22 - g12^2, ...
            c00 = fmsub(g[1, 1], g[2, 2], g[1, 2], g[1, 2], "c00")
            c01 = fmsub(g[0, 2], g[1, 2], g[0, 1], g[2, 2], "c01")
            c02 = fmsub(g[0, 1], g[1, 2], g[0, 2], g[1, 1], "c02")
            c11 = fmsub(g[0, 0], g[2, 2], g[0, 2], g[0, 2], "c11")
            c12 = fmsub(g[0, 1], g[0, 2], g[0, 0], g[1, 2], "c12")
            c22 = fmsub(g[0, 0], g[1, 1], g[0, 1], g[0, 1], "c22")

            # det = g00*c00 + g01*c01 + g02*c02
            d0 = tt(g[0, 0], c00, op.mult, "d0")
            d1 = tt(g[0, 1], c01, op.mult, "d1")
            d2 = tt(g[0, 2], c02, op.mult, "d2")
            det = tt(d0, d1, op.add, "deta")
            det = tt(det, d2, op.add, "det")
            rdet = coef.tile([P, PL], f32, tag="rdet")
            nc.vector.reciprocal(rdet[:], det[:])

            # Ginv rows stacked as [P, 3(j), PL]
            IV = [coef.tile([P, 3, PL], f32, tag=f"IV{k}", name=f"IV{k}")
                  for k in range(3)]
            for (i, j), cof in [((0, 0), c00), ((0, 1), c01), ((0, 2), c02),
                                ((1, 1), c11), ((1, 2), c12), ((2, 2), c22)]:
                nc.vector.tensor_tensor(IV[i][:, j, :], cof[:], rdet[:],
                                        op.mult)
                if i != j:
                    nc.scalar.copy(IV[j][:, i, :], IV[i][:, j, :])

            # A-row i = sum_{k<=i} bL_ik (broadcast over j) * IV_k
            def blb(i, k):
                return L[(i, k)][:].rearrange(
                    "p l -> p () l").broadcast_to([P, 3, PL])

            AR = []
            for i in range(3):
                ar = coef.tile([P, 3, PL], f32, tag=f"AR{i}", name=f"AR{i}")
                if i == 0:
                    nc.vector.tensor_tensor(ar[:], IV[0][:], blb(0, 0),
                                            op.mult)
                else:
                    acc = tmp.tile([P, 3, PL], f32, tag="Aacc")
                    nc.vector.tensor_tensor(acc[:], IV[0][:], blb(i, 0),
                                            op.mult)
                    for k in range(1, i + 1):
                        pr = tmp.tile([P, 3, PL], f32, tag="Apr")
                        nc.vector.tensor_tensor(pr[:], IV[k][:], blb(i, k),
                                                op.mult)
                        dst = ar if k == i else tmp.tile([P, 3, PL], f32,
                                                         tag="Aacc")
                        nc.vector.tensor_tensor(dst[:], acc[:], pr[:], op.add)
                        acc = dst
                AR.append(ar)
            eAR = []
            for i in range(3):
                e = coef.tile([P, 3, PL], f16, tag=f"eAR{i}", name=f"eAR{i}")
                nc.scalar.copy(e[:], AR[i][:])
                eAR.append(e)
            eA = {(i, j): eAR[i][:, j:j + 1, :]
                  for i in range(3) for j in range(3)}

            mtmp = stack.enter_context(tc.tile_pool(name="mtmp", bufs=2))
            outs = stack.enter_context(tc.tile_pool(name="outs", bufs=2))
            psum = stack.enter_context(
                tc.tile_pool(name="psum", bufs=2, space="PSUM"))

            # ------------- stage C: batched main stage ---------------------
            # products on DVE/GP; channel sums accumulated on PE via
            # identity matmuls into PSUM; the dr BG*r term rides the same
            # accumulation through a BG-scaled identity; ACT drains with the
            # channel sign.
            def products(coeffs, ins, bh, engine):
                bsl = slice(bh * HB, (bh + 1) * HB)
                prs = []
                for idx, (cc, dd) in enumerate(zip(coeffs, ins)):
                    pr = mtmp.tile([P, HB, PL], f16, tag=f"pr{idx}", bufs=3)
                    bc = cc.broadcast_to([P, HB, PL])
                    engine.tensor_tensor(pr[:], dd[:, bsl, :], bc, op.mult)
                    prs.append(pr)
                return prs

            def accum_psum(prs, bh, extra_rhs=None):
                # extra_rhs: (data_tile, weights) accumulated with `weights`
                bsl = slice(bh * HB, (bh + 1) * HB)
                n = len(prs) + (1 if extra_rhs is not None else 0)
                pss = []
                for bq in range(2):
                    ps = psum.tile([P, 1024], f32, tag="ps", bufs=4)
                    for s2 in range(2):
                        sl = slice(s2 * 512, (s2 + 1) * 512)
                        gl = slice(bq * 1024 + s2 * 512,
                                   bq * 1024 + (s2 + 1) * 512)
                        idx = 0
                        for pr in prs:
                            rhs = pr[:].rearrange("p b l -> p (b l)")
                            nc.tensor.matmul(
                                ps[:, sl], ident[:], rhs[:, gl],
                                start=(idx == 0), stop=(idx == n - 1))
                            idx += 1
                        if extra_rhs is not None:
                            dd, wt = extra_rhs
                            rhs = dd[:, bsl, :].rearrange("p b l -> p (b l)")
                            qsl = slice(bh * 2048 + bq * 1024 + s2 * 512,
                                        bh * 2048 + bq * 1024 + (s2 + 1) * 512)
                            full = dd[:].rearrange("p b l -> p (b l)")
                            nc.tensor.matmul(
                                ps[:, sl], wt, full[:, qsl],
                                start=(idx == 0), stop=(idx == n - 1))
                            idx += 1
                    pss.append(ps)
                return pss

            def drain(pss, out_tile, scale=1.0):
                for bq, ps in enumerate(pss):
                    nc.scalar.mul(
                        out_tile[:, bq * QB:(bq + 1) * QB, :],
                        ps[:].rearrange("p (b l) -> p b l", b=QB), scale)

            def emit_dx(i, bh):
                bsl = slice(bh * HB, (bh + 1) * HB)
                dx = outs.tile([P, HB, PL], f16, tag=f"dx{i}", name=f"dx{i}")
                coeffs = [eA[(i, 0)], eA[(i, 1)], eA[(i, 2)]]
                prs = products(coeffs, r_t, bh, nc.vector)
                if DX_PE[i]:
                    pss = accum_psum(prs, bh)
                    drain(pss, dx)
                else:
                    s01 = mtmp.tile([P, HB, PL], f16, tag="s01")
                    nc.vector.tensor_tensor(s01[:], prs[0][:], prs[1][:],
                                            op.add)
                    nc.vector.tensor_tensor(dx[:], s01[:], prs[2][:], op.add)
                nc.sync.dma_start(out=drift_o[i, :, bsl, :], in_=dx[:])

            def emit_dr(i, bh):
                bsl = slice(bh * HB, (bh + 1) * HB)
                dr = outs.tile([P, HB, PL], f16, tag=f"dr{i}", name=f"dr{i}")
                coeffs = [eL[(i, j)] for j in range(i + 1)]
                eng = nc.gpsimd if (GP_DR2 and i == 2) else nc.vector
                prs = products(coeffs, x_t, bh, eng)
                if DR_PE[i]:
                    pss = accum_psum(prs, bh, extra_rhs=(r_t[i], bgid[:]))
                    drain(pss, dr, scale=-1.0)
                else:
                    acc = prs[0]
                    for k in range(1, len(prs)):
                        nxt = mtmp.tile([P, HB, PL], f16, tag="racc")
                        nc.vector.tensor_tensor(nxt[:], acc[:], prs[k][:],
                                                op.add)
                        acc = nxt
                    negr = mtmp.tile([P, HB, PL], f16, tag="negr")
                    nc.vector.tensor_scalar(negr[:], r_t[i][:, bsl, :], -BG,
                                            None, op.mult)
                    nc.vector.tensor_tensor(dr[:], negr[:], acc[:],
                                            op.subtract)
                nc.sync.dma_start(out=drift_o[C + i, :, bsl, :], in_=dr[:])

            # dr first (needs only L), dx after (needs A); dr0/dx2 are
            # DVE-assembled and go last so the PE/ACT pipelines stay fed.
            for bh in range(2):
                emit_dr(2, bh)
                emit_dr(1, bh)
                emit_dx(0, bh)
                emit_dx(1, bh)
                emit_dx(2, bh)
                emit_dr(0, bh)

    nc.compile()
    return nc


def _host_reference(u, score_x, t):
    """Pure-numpy fallback (exact reference math); used only when t[0]==1.0
    (the stateful normalization branch, never hit with uniform t)."""
    x, r = u[:, :C], u[:, C:]
    s = np.transpose(score_x, (0, 2, 3, 1)).astype(np.float32)
    G = np.einsum("bhwi,bhwj->hwij", s, s) / np.float32(score_x.shape[0])
    t0 = t[0]
    diag_mean = np.mean(np.trace(G, axis1=-2, axis2=-1)) / C
    normalization = np.where(t0 == 1.0, diag_mean * M_INV, 1.0)
    G = G / normalization
    G_id = (1.0 / M_INV) * np.eye(C, dtype=G.dtype)
    alpha = RIEMANN_MIX * np.exp(-K_DECAY * (1.0 - t0))
    G = alpha * G + (1.0 - alpha) * G_id
    G_inv = np.linalg.inv(G).astype(np.float32)
    G_sqrt = np.linalg.cholesky(G).astype(np.float32)

    def mm(Amat, Bf):
        return np.einsum("hwij,bjhw->bihw", Amat, Bf).astype(np.float32)

    hr = mm(G_inv, r)
    drift_x = BETA_C * mm(G_sqrt, hr)
    drift_r = (-BETA_C * mm(G_sqrt, x) - BETA_C * GAMMA_C * mm(G, hr))
    diffusion_x = np.zeros_like(x)
    diffusion_r = BG_SCALE * mm(G_sqrt, np.ones_like(r))
    drift = np.concatenate((drift_x, drift_r), axis=1)
    diffusion = np.concatenate((diffusion_x, diffusion_r), axis=1)
    return drift, diffusion


def kernel(u, score_x, t):
    from concourse.bass_utils import run_bass_kernel_spmd

    u = np.ascontiguousarray(np.asarray(u, dtype=np.float32))
    score_x = np.ascontiguousarray(np.asarray(score_x, dtype=np.float32))
    t = np.asarray(t, dtype=np.float32)

    t0 = float(t[0])
    if t0 == 1.0:
        return _host_reference(u, score_x, t)

    alpha = RIEMANN_MIX * math.exp(-K_DECAY * (1.0 - t0))
    ca = alpha / B          # normalization == 1.0 on this branch
    cid = (1.0 - alpha) / M_INV

    key = (round(ca, 12), round(cid, 12))
    nc = _PROG_CACHE.get(key)
    if nc is None:
        nc = _build_program(ca, cid)
        _PROG_CACHE[key] = nc

    in_maps = []
    for k in range(N_CORES):
        rows = slice(k * ROWS, (k + 1) * ROWS)
        s_np = (score_x[:, :, rows, :]
                .reshape(B, C, P, PL).transpose(1, 2, 0, 3)
                .astype(np.float16))
        u_np = (u[:, :, rows, :]
                .reshape(B, 2 * C, P, PL).transpose(1, 2, 0, 3)
                .astype(np.float16))
        in_maps.append({
            "s_in": np.ascontiguousarray(s_np),
            "u_in": np.ascontiguousarray(u_np),
            "ident": np.eye(P, dtype=np.float16),
        })

    res = run_bass_kernel_spmd(nc, in_maps, list(range(N_CORES)))

    drift = np.empty((B, 2 * C, HW, HW), dtype=np.float32)
    diffusion = np.zeros((B, 2 * C, HW, HW), dtype=np.float32)
    for k in range(N_CORES):
        rows = slice(k * ROWS, (k + 1) * ROWS)
        dk = res.results[k]["drift"].astype(np.float32)     # [6, P, B, PL]
        drift[:, :, rows, :] = dk.transpose(2, 0, 1, 3).reshape(
            B, 2 * C, ROWS, HW)
        difk = res.results[k]["dif"].reshape(C, ROWS, HW)   # [3, P, PL]
        diffusion[:, C:, rows, :] = difk[None, :, :, :]
    return drift, diffusion


# revision 9
# speedup vs baseline: 1.2820x; 1.2820x over previous
"""CLD sde_reverse (Riemann geometry) Trainium2 kernel — v2.

Contract: kernel(u, score_x, t) -> (drift, diffusion), full (unsharded) numpy
arrays, computed on 8 NeuronCores via bass/Tile + run_bass_kernel_spmd.

Sharding: pixels (image rows) are sharded 8 ways; every core sees all 64 batch
elements for its 32 rows. All math is per-pixel 3x3 — no collectives.

v2 design (from HW microbenchmarks):
  - DVE tensor_tensor runs at 1x (0.56 ns/elem) regardless of dtype; the
    baseline was DVE-bound at ~94% busy.  v2 load-balances the big batched
    elementwise work across DVE / GpSimd / ACT / PE:
      * products (coef x data) on DVE, a few on GpSimd (1.93 ns/elem)
      * per-channel sums accumulated on the PE via identity matmuls
        (379 ns / 512-free matmul sustained), drained to fp16 by ACT
      * the dr BG*r term rides the same PSUM accumulation through a
        BG-scaled identity weight tile; the drain applies the -1 sign
      * stage A batch-reduction: PE chunk-accumulation (8 matmuls/term)
        for most terms + short fp16 fold chains on DVE for the rest
      * squares on ACT, crosses on DVE/GpSimd
  - 3x3 chol/inverse stays on small [P,64] planes; ACT Sqrt used directly
    (no Newton step).

Device layout per core: pixel p in [0,8192) maps to (part, pl) = (p>>6, p&63);
tensors are [channel, 128 part, 64 batch, 64 pl] so every DMA run is
contiguous.
"""

import math

import numpy as np

# ---- model constants (from the reference config) ----
M_INV = 4.0
GAMMA_BIG = 0.04
BETA0 = 4.0
RIEMANN_MIX = 0.5
K_DECAY = 4.5
C = 3
HW = 256
B = 64

N_CORES = 8
ROWS = HW // N_CORES  # 32 rows per core
P = 128               # SBUF partitions
PL = (ROWS * HW) // P  # 64 free pixels per partition

BETA_C = BETA0 * math.sqrt(M_INV)        # 8.0
GAMMA_C = GAMMA_BIG * math.sqrt(M_INV)   # 0.08
BG = BETA_C * GAMMA_C                    # 0.64
BG_SCALE = math.sqrt(2.0 * BETA_C * GAMMA_C)

_PROG_CACHE: dict = {}

# ---- engine assignment (tuned against NTFF profiles) ----
# GpSimd is unusable here: a concurrent GpSimd tensor_tensor starves the DVE
# (measured 3.3x slowdown on overlapping DVE ops), so everything elementwise
# stays on DVE/ACT and sums go to the PE.
PE_REDUCE = {(0, 0), (1, 1)}   # stage-A terms reduced on PE (rest: DVE folds)
DX_PE = (True, True, False)  # dx channel sums on PE (else DVE adds)
DR_PE = (False, True, True)  # dr channel assembly on PE (else DVE TS+sub)


def _build_program(ca: float, cid: float, n_reps: int = 1):
    """Build + compile the per-core SPMD bass program.

    ca  = alpha / (B * normalization)   (scale for the raw sum S_ij)
    cid = (1 - alpha) / M_INV           (identity mixture term)
    """
    from contextlib import ExitStack

    import concourse.bacc as bacc
    import concourse.mybir as mybir
    import concourse.tile as tile

    dt = mybir.dt
    op = mybir.AluOpType
    f32 = dt.float32
    f16 = dt.float16
    AF = mybir.ActivationFunctionType

    nc = bacc.Bacc("TRN2", target_bir_lowering=False, debug=False,
                   num_devices=N_CORES)

    s_in = nc.dram_tensor("s_in", [C, P, B, PL], f16,
                          kind="ExternalInput").ap()
    u_in = nc.dram_tensor("u_in", [2 * C, P, B, PL], f16,
                          kind="ExternalInput").ap()
    id_in = nc.dram_tensor("ident", [P, P], dt.float16,
                           kind="ExternalInput").ap()
    drift_o = nc.dram_tensor("drift", [2 * C, P, B, PL], f16,
                             kind="ExternalOutput").ap()
    dif_o = nc.dram_tensor("dif", [C, P, PL], f32, kind="ExternalOutput").ap()

    HB = B // 2   # batch half
    QB = B // 4   # batch quarter

    with tile.TileContext(nc) as tc:
      for _rep in range(n_reps):
        with ExitStack() as stack:
            coef = stack.enter_context(tc.tile_pool(name="coef", bufs=1))
            data = stack.enter_context(tc.tile_pool(name="data", bufs=1))
            tmp = stack.enter_context(tc.tile_pool(name="tmp", bufs=2))
            ident = coef.tile([P, P], f16, tag="ident")
            bgid = coef.tile([P, P], f16, tag="bgid")

            g = {}
            with tc.tile_pool(name="score", bufs=1) as score_pool, \
                 tc.tile_pool(name="prod", bufs=1) as prod_pool, \
                 tc.tile_pool(name="psA", bufs=1, space="PSUM") as psA:
                # ---------------- input DMA ----------------
                s_t = [score_pool.tile([P, B, PL], f16, tag=f"s{c}",
                                       name=f"s{c}") for c in range(C)]
                for bh in range(2):
                    bsl = slice(bh * HB, (bh + 1) * HB)
                    for c in range(C):
                        nc.sync.dma_start(out=s_t[c][:, bsl, :],
                                          in_=s_in[c, :, bsl, :])
                nc.sync.dma_start(out=ident[:], in_=id_in[:])
                nc.scalar.mul(bgid[:], ident[:], BG)
                # u: x channels first (dr path starts earlier), then r
                u_t = []
                for c in range(2 * C):
                    ut = data.tile([P, B, PL], f16, tag=f"u{c}")
                    for bh in range(2):
                        bsl = slice(bh * HB, (bh + 1) * HB)
                        nc.sync.dma_start(out=ut[:, bsl, :],
                                          in_=u_in[c, :, bsl, :])
                    u_t.append(ut)
                x_t, r_t = u_t[:C], u_t[C:]

                # ---------------- stage A ----------------
                # products: squares on ACT, crosses on DVE/GP (b-halved so
                # work starts as soon as the first half-DMAs land)
                q = {}
                for i in range(C):
                    sq = prod_pool.tile([P, B, PL], f16, tag=f"sq{i}")
                    for bh in range(2):
                        bsl = slice(bh * HB, (bh + 1) * HB)
                        nc.scalar.activation(sq[:, bsl, :], s_t[i][:, bsl, :],
                                             AF.Square)
                    q[(i, i)] = sq
                for (i, j) in [(0, 1), (0, 2), (1, 2)]:
                    qt = prod_pool.tile([P, B, PL], f16, tag=f"q{i}{j}")
                    for bh in range(2):
                        bsl = slice(bh * HB, (bh + 1) * HB)
                        nc.vector.tensor_tensor(qt[:, bsl, :],
                                                s_t[i][:, bsl, :],
                                                s_t[j][:, bsl, :], op.mult)
                    q[(i, j)] = qt

                # reductions over batch -> S planes [P, PL] f32-ish
                S = {}
                for key in [(0, 0), (1, 1), (2, 2), (0, 1), (0, 2), (1, 2)]:
                    qt = q[key]
                    if key in PE_REDUCE:
                        ps = psA.tile([P, 8 * PL], f32, tag=f"psA{key[0]}{key[1]}")
                        for ck in range(8):
                            rhs = qt[:, ck * 8:(ck + 1) * 8, :].rearrange(
                                "p b l -> p (b l)")
                            nc.tensor.matmul(ps[:], ident[:], rhs,
                                             start=(ck == 0), stop=(ck == 7))
                        r8 = tmp.tile([P, 8, PL], f32, tag="r8")
                        nc.scalar.copy(
                            r8[:], ps[:].rearrange("p (b l) -> p b l", b=8))
                        r4 = tmp.tile([P, 4, PL], f32, tag="r4")
                        nc.vector.tensor_tensor(r4[:], r8[:, 0:4, :],
                                                r8[:, 4:8, :], op.add)
                        r2 = tmp.tile([P, 2, PL], f32, tag="r2")
                        nc.vector.tensor_tensor(r2[:], r4[:, 0:2, :],
                                                r4[:, 2:4, :], op.add)
                        st = tmp.tile([P, PL], f32, tag=f"S{key[0]}{key[1]}")
                        nc.vector.tensor_tensor(st[:], r2[:, 0, :],
                                                r2[:, 1, :], op.add)
                        S[key] = st
                    else:
                        # fp16 fold chain 64 -> 2 on DVE, final add emits f32
                        cur = qt
                        w = B
                        while w > 2:
                            h = w // 2
                            nxt = tmp.tile([P, h, PL], f16, tag=f"f{h}",
                                           bufs=3)
                            nc.vector.tensor_tensor(
                                nxt[:], cur[:, 0:h, :], cur[:, h:w, :],
                                op.add)
                            cur = nxt
                            w = h
                        st = tmp.tile([P, PL], f32, tag=f"S{key[0]}{key[1]}")
                        nc.vector.tensor_tensor(st[:], cur[:, 0, :],
                                                cur[:, 1, :], op.add)
                        S[key] = st

                for i in range(C):
                    gii = coef.tile([P, PL], f32, tag=f"g{i}{i}")
                    nc.scalar.activation(gii[:], S[(i, i)][:], AF.Copy,
                                         bias=float(cid), scale=float(ca))
                    g[(i, i)] = gii
                for (i, j) in [(0, 1), (0, 2), (1, 2)]:
                    gij = coef.tile([P, PL], f32, tag=f"g{i}{j}")
                    nc.scalar.mul(gij[:], S[(i, j)][:], float(ca))
                    g[(i, j)] = gij
                    g[(j, i)] = gij

            # ------------- stage B: per-pixel 3x3 coefficients -------------
            def tt(a, b_, o, tag):
                t = coef.tile([P, PL], f32, tag=tag)
                nc.vector.tensor_tensor(t[:], a[:], b_[:], o)
                return t

            def fmsub(a, b_, c_, d_, tag):
                # a*b - c*d
                t1 = tmp.tile([P, PL], f32, tag="fm1")
                nc.vector.tensor_tensor(t1[:], a[:], b_[:], op.mult)
                t2 = tmp.tile([P, PL], f32, tag="fm2")
                nc.vector.tensor_tensor(t2[:], c_[:], d_[:], op.mult)
                t = coef.tile([P, PL], f32, tag=tag)
                nc.vector.tensor_tensor(t[:], t1[:], t2[:], op.subtract)
                return t

            def to16(plane, tag):
                e = coef.tile([P, 1, PL], f16, tag=tag)
                nc.scalar.copy(e[:, 0, :], plane[:])
                return e

            def sqrt_p(a, tag):
                out = coef.tile([P, PL], f32, tag=tag)
                nc.scalar.activation(out[:], a[:], AF.Sqrt)
                return out

            l00 = sqrt_p(g[0, 0], "l00")
            il00 = coef.tile([P, PL], f32, tag="il00")
            nc.vector.reciprocal(il00[:], l00[:])
            l10 = tt(g[0, 1], il00, op.mult, "l10")
            l20 = tt(g[0, 2], il00, op.mult, "l20")
            t = tt(l10, l10, op.mult, "l10sq")
            dd1 = tt(g[1, 1], t, op.subtract, "dd1")
            l11 = sqrt_p(dd1, "l11")
            il11 = coef.tile([P, PL], f32, tag="il11")
            nc.vector.reciprocal(il11[:], l11[:])
            t = tt(l20, l10, op.mult, "l20l10")
            t = tt(g[1, 2], t, op.subtract, "g12m")
            l21 = tt(t, il11, op.mult, "l21")
            t = tt(l20, l20, op.mult, "l20sq")
            dd2 = tt(g[2, 2], t, op.subtract, "dd2a")
            t = tt(l21, l21, op.mult, "l21sq")
            dd2 = tt(dd2, t, op.subtract, "dd2")
            l22 = sqrt_p(dd2, "l22")

            # bL = beta * L  (scaled once, reused by drift_r, A, diffusion)
            L = {}
            for (i, j), lt in [((0, 0), l00), ((1, 0), l10), ((1, 1), l11),
                               ((2, 0), l20), ((2, 1), l21), ((2, 2), l22)]:
                blt = coef.tile([P, PL], f32, tag=f"bl{i}{j}")
                nc.scalar.mul(blt[:], lt[:], BETA_C)
                L[(i, j)] = blt
            eL = {(i, j): to16(L[(i, j)], f"eL{i}{j}")[:]
                  for (i, j) in [(0, 0), (1, 0), (1, 1),
                                 (2, 0), (2, 1), (2, 2)]}

            # diffusion_r rows (batch-independent): bg/beta * row sums of bL
            bgob = BG_SCALE / BETA_C
            dif0 = coef.tile([P, PL], f32, tag="dif0")
            nc.scalar.mul(dif0[:], L[0, 0][:], bgob)
            t = tt(L[1, 0], L[1, 1], op.add, "difs1")
            dif1 = coef.tile([P, PL], f32, tag="dif1")
            nc.scalar.mul(dif1[:], t[:], bgob)
            t = tt(L[2, 0], L[2, 1], op.add, "difs2a")
            t = tt(t, L[2, 2], op.add, "difs2")
            dif2 = coef.tile([P, PL], f32, tag="dif2")
            nc.scalar.mul(dif2[:], t[:], bgob)
            for i, dtile in enumerate((dif0, dif1, dif2)):
                nc.sync.dma_start(out=dif_o[i], in_=dtile[:])

            # adjugate (symmetric): c00 = g11*g22 - g12^2, ...
            c00 = fmsub(g[1, 1], g[2, 2], g[1, 2], g[1, 2], "c00")
            c01 = fmsub(g[0, 2], g[1, 2], g[0, 1], g[2, 2], "c01")
            c02 = fmsub(g[0, 1], g[1, 2], g[0, 2], g[1, 1], "c02")
            c11 = fmsub(g[0, 0], g[2, 2], g[0, 2], g[0, 2], "c11")
            c12 = fmsub(g[0, 1], g[0, 2], g[0, 0], g[1, 2], "c12")
            c22 = fmsub(g[0, 0], g[1, 1], g[0, 1], g[0, 1], "c22")

            # det = g00*c00 + g01*c01 + g02*c02
            d0 = tt(g[0, 0], c00, op.mult, "d0")
            d1 = tt(g[0, 1], c01, op.mult, "d1")
            d2 = tt(g[0, 2], c02, op.mult, "d2")
            det = tt(d0, d1, op.add, "deta")
            det = tt(det, d2, op.add, "det")
            rdet = coef.tile([P, PL], f32, tag="rdet")
            nc.vector.reciprocal(rdet[:], det[:])

            # Ginv rows stacked as [P, 3(j), PL]
            IV = [coef.tile([P, 3, PL], f32, tag=f"IV{k}", name=f"IV{k}")
                  for k in range(3)]
            for (i, j), cof in [((0, 0), c00), ((0, 1), c01), ((0, 2), c02),
                                ((1, 1), c11), ((1, 2), c12), ((2, 2), c22)]:
                nc.vector.tensor_tensor(IV[i][:, j, :], cof[:], rdet[:],
                                        op.mult)
                if i != j:
                    nc.scalar.copy(IV[j][:, i, :], IV[i][:, j, :])

            # A-row i = sum_{k<=i} bL_ik (broadcast over j) * IV_k
            def blb(i, k):
                return L[(i, k)][:].rearrange(
                    "p l -> p () l").broadcast_to([P, 3, PL])

            AR = []
            for i in range(3):
                ar = coef.tile([P, 3, PL], f32, tag=f"AR{i}", name=f"AR{i}")
                if i == 0:
                    nc.vector.tensor_tensor(ar[:], IV[0][:], blb(0, 0),
                                            op.mult)
                else:
                    acc = tmp.tile([P, 3, PL], f32, tag="Aacc")
                    nc.vector.tensor_tensor(acc[:], IV[0][:], blb(i, 0),
                                            op.mult)
                    for k in range(1, i + 1):
                        pr = tmp.tile([P, 3, PL], f32, tag="Apr")
                        nc.vector.tensor_tensor(pr[:], IV[k][:], blb(i, k),
                                                op.mult)
                        dst = ar if k == i else tmp.tile([P, 3, PL], f32,
                                                         tag="Aacc")
                        nc.vector.tensor_tensor(dst[:], acc[:], pr[:], op.add)
                        acc = dst
                AR.append(ar)
            eAR = []
            for i in range(3):
                e = coef.tile([P, 3, PL], f16, tag=f"eAR{i}", name=f"eAR{i}")
                nc.scalar.copy(e[:], AR[i][:])
                eAR.append(e)
            eA = {(i, j): eAR[i][:, j:j + 1, :]
                  for i in range(3) for j in range(3)}

            mtmp = stack.enter_context(tc.tile_pool(name="mtmp", bufs=2))
            outs = stack.enter_context(tc.tile_pool(name="outs", bufs=1))
            psum = stack.enter_context(
                tc.tile_pool(name="psum", bufs=2, space="PSUM"))

            # ------------- stage C: batched main stage ---------------------
            # full-batch products on DVE; channel sums accumulated on PE via
            # identity matmuls into PSUM (term-major so weights stay loaded);
            # the dr BG*r term rides the same accumulation through a
            # BG-scaled identity; ACT drains with the channel sign and the
            # output DMA streams per batch-quarter.
            def products(coeffs, ins):
                prs = []
                for idx, (cc, dd) in enumerate(zip(coeffs, ins)):
                    pr = mtmp.tile([P, B, PL], f16, tag=f"pr{idx}", bufs=2)
                    bc = cc.broadcast_to([P, B, PL])
                    nc.vector.tensor_tensor(pr[:], dd[:], bc, op.mult)
                    prs.append(pr)
                return prs

            def pe_channel(tag, prs, extra_rhs, scale, dma_out):
                # rhs list: (flat AP, weights) pairs
                rhss = [(pr[:].rearrange("p b l -> p (b l)"), ident[:])
                        for pr in prs]
                if extra_rhs is not None:
                    rhss.append((extra_rhs[:].rearrange("p b l -> p (b l)"),
                                 bgid[:]))
                n = len(rhss)
                for bq in range(4):
                    ps = psum.tile([P, 1024], f32, tag="ps", bufs=4)
                    for idx, (rhs, wt) in enumerate(rhss):
                        for s2 in range(2):
                            sl = slice(s2 * 512, (s2 + 1) * 512)
                            gl = slice(bq * 1024 + s2 * 512,
                                       bq * 1024 + (s2 + 1) * 512)
                            nc.tensor.matmul(
                                ps[:, sl], wt, rhs[:, gl],
                                start=(idx == 0), stop=(idx == n - 1))
                    qsl = slice(bq * 16, (bq + 1) * 16)
                    qo = outs.tile([P, 16, PL], f16, tag="qo", bufs=4)
                    nc.scalar.mul(
                        qo[:], ps[:].rearrange("p (b l) -> p b l", b=16),
                        scale)
                    nc.sync.dma_start(out=dma_out[:, qsl, :], in_=qo[:])

            def emit_dx(i):
                coeffs = [eA[(i, 0)], eA[(i, 1)], eA[(i, 2)]]
                prs = products(coeffs, r_t)
                if DX_PE[i]:
                    pe_channel(f"dx{i}", prs, None, 1.0, drift_o[i])
                else:
                    dx = outs.tile([P, B, PL], f16, tag="full", bufs=2)
                    s01 = mtmp.tile([P, B, PL], f16, tag="s01", bufs=1)
                    nc.vector.tensor_tensor(s01[:], prs[0][:], prs[1][:],
                                            op.add)
                    nc.vector.tensor_tensor(dx[:], s01[:], prs[2][:], op.add)
                    nc.sync.dma_start(out=drift_o[i], in_=dx[:])

            def emit_dr(i):
                coeffs = [eL[(i, j)] for j in range(i + 1)]
                prs = products(coeffs, x_t)
                if DR_PE[i]:
                    pe_channel(f"dr{i}", prs, r_t[i], -1.0, drift_o[C + i])
                else:
                    dr = outs.tile([P, B, PL], f16, tag="full", bufs=2)
                    acc = prs[0]
                    for k in range(1, len(prs)):
                        nxt = mtmp.tile([P, B, PL], f16, tag="racc", bufs=1)
                        nc.vector.tensor_tensor(nxt[:], acc[:], prs[k][:],
                                                op.add)
                        acc = nxt
                    negr = mtmp.tile([P, B, PL], f16, tag="negr", bufs=1)
                    nc.vector.tensor_scalar(negr[:], r_t[i][:], -BG,
                                            None, op.mult)
                    nc.vector.tensor_tensor(dr[:], negr[:], acc[:],
                                            op.subtract)
                    nc.sync.dma_start(out=drift_o[C + i], in_=dr[:])

            # dr first (needs only L), dx after (needs A); dr0/dx2 are
            # DVE-assembled and go last so the PE/ACT pipelines stay fed.
            emit_dr(2)
            emit_dr(1)
            emit_dx(0)
            emit_dx(1)
            emit_dx(2)
            emit_dr(0)

    nc.compile()
    return nc


def _host_reference(u, score_x, t):
    """Pure-numpy fallback (exact reference math); used only when t[0]==1.0
    (the stateful normalization branch, never hit with uniform t)."""
    x, r = u[:, :C], u[:, C:]
    s = np.transpose(score_x, (0, 2, 3, 1)).astype(np.float32)
    G = np.einsum("bhwi,bhwj->hwij", s, s) / np.float32(score_x.shape[0])
    t0 = t[0]
    diag_mean = np.mean(np.trace(G, axis1=-2, axis2=-1)) / C
    normalization = np.where(t0 == 1.0, diag_mean * M_INV, 1.0)
    G = G / normalization
    G_id = (1.0 / M_INV) * np.eye(C, dtype=G.dtype)
    alpha = RIEMANN_MIX * np.exp(-K_DECAY * (1.0 - t0))
    G = alpha * G + (1.0 - alpha) * G_id
    G_inv = np.linalg.inv(G).astype(np.float32)
    G_sqrt = np.linalg.cholesky(G).astype(np.float32)

    def mm(Amat, Bf):
        return np.einsum("hwij,bjhw->bihw", Amat, Bf).astype(np.float32)

    hr = mm(G_inv, r)
    drift_x = BETA_C * mm(G_sqrt, hr)
    drift_r = (-BETA_C * mm(G_sqrt, x) - BETA_C * GAMMA_C * mm(G, hr))
    diffusion_x = np.zeros_like(x)
    diffusion_r = BG_SCALE * mm(G_sqrt, np.ones_like(r))
    drift = np.concatenate((drift_x, drift_r), axis=1)
    diffusion = np.concatenate((diffusion_x, diffusion_r), axis=1)
    return drift, diffusion


def kernel(u, score_x, t):
    from concourse.bass_utils import run_bass_kernel_spmd

    u = np.ascontiguousarray(np.asarray(u, dtype=np.float32))
    score_x = np.ascontiguousarray(np.asarray(score_x, dtype=np.float32))
    t = np.asarray(t, dtype=np.float32)

    t0 = float(t[0])
    if t0 == 1.0:
        return _host_reference(u, score_x, t)

    alpha = RIEMANN_MIX * math.exp(-K_DECAY * (1.0 - t0))
    ca = alpha / B          # normalization == 1.0 on this branch
    cid = (1.0 - alpha) / M_INV

    key = (round(ca, 12), round(cid, 12))
    nc = _PROG_CACHE.get(key)
    if nc is None:
        nc = _build_program(ca, cid)
        _PROG_CACHE[key] = nc

    in_maps = []
    for k in range(N_CORES):
        rows = slice(k * ROWS, (k + 1) * ROWS)
        s_np = (score_x[:, :, rows, :]
                .reshape(B, C, P, PL).transpose(1, 2, 0, 3)
                .astype(np.float16))
        u_np = (u[:, :, rows, :]
                .reshape(B, 2 * C, P, PL).transpose(1, 2, 0, 3)
                .astype(np.float16))
        in_maps.append({
            "s_in": np.ascontiguousarray(s_np),
            "u_in": np.ascontiguousarray(u_np),
            "ident": np.eye(P, dtype=np.float16),
        })

    res = run_bass_kernel_spmd(nc, in_maps, list(range(N_CORES)))

    drift = np.empty((B, 2 * C, HW, HW), dtype=np.float32)
    diffusion = np.zeros((B, 2 * C, HW, HW), dtype=np.float32)
    for k in range(N_CORES):
        rows = slice(k * ROWS, (k + 1) * ROWS)
        dk = res.results[k]["drift"].astype(np.float32)     # [6, P, B, PL]
        drift[:, :, rows, :] = dk.transpose(2, 0, 1, 3).reshape(
            B, 2 * C, ROWS, HW)
        difk = res.results[k]["dif"].reshape(C, ROWS, HW)   # [3, P, PL]
        diffusion[:, C:, rows, :] = difk[None, :, :, :]
    return drift, diffusion


# revision 11
# speedup vs baseline: 6.2045x; 4.8395x over previous
"""CLD sde_reverse (Riemann geometry) Trainium2 kernel — v2.

Contract: kernel(u, score_x, t) -> (drift, diffusion), full (unsharded) numpy
arrays, computed on 8 NeuronCores via bass/Tile + run_bass_kernel_spmd.

Sharding: pixels (image rows) are sharded 8 ways; every core sees all 64 batch
elements for its 32 rows. All math is per-pixel 3x3 — no collectives.

v2 design (from HW microbenchmarks):
  - DVE tensor_tensor runs at 1x (0.56 ns/elem) regardless of dtype; the
    baseline was DVE-bound at ~94% busy.  v2 load-balances the big batched
    elementwise work across DVE / GpSimd / ACT / PE:
      * products (coef x data) on DVE, a few on GpSimd (1.93 ns/elem)
      * per-channel sums accumulated on the PE via identity matmuls
        (379 ns / 512-free matmul sustained), drained to fp16 by ACT
      * the dr BG*r term rides the same PSUM accumulation through a
        BG-scaled identity weight tile; the drain applies the -1 sign
      * stage A batch-reduction: PE chunk-accumulation (8 matmuls/term)
        for most terms + short fp16 fold chains on DVE for the rest
      * squares on ACT, crosses on DVE/GpSimd
  - 3x3 chol/inverse stays on small [P,64] planes; ACT Sqrt used directly
    (no Newton step).

Device layout per core: pixel p in [0,8192) maps to (part, pl) = (p>>6, p&63);
tensors are [channel, 128 part, 64 batch, 64 pl] so every DMA run is
contiguous.
"""

import math

import numpy as np

# ---- model constants (from the reference config) ----
M_INV = 4.0
GAMMA_BIG = 0.04
BETA0 = 4.0
RIEMANN_MIX = 0.5
K_DECAY = 4.5
C = 3
HW = 256
B = 64

N_CORES = 8
ROWS = HW // N_CORES  # 32 rows per core
P = 128               # SBUF partitions
PL = (ROWS * HW) // P  # 64 free pixels per partition

BETA_C = BETA0 * math.sqrt(M_INV)        # 8.0
GAMMA_C = GAMMA_BIG * math.sqrt(M_INV)   # 0.08
BG = BETA_C * GAMMA_C                    # 0.64
BG_SCALE = math.sqrt(2.0 * BETA_C * GAMMA_C)

_PROG_CACHE: dict = {}

# ---- engine assignment (tuned against NTFF profiles) ----
# GpSimd is unusable here: a concurrent GpSimd tensor_tensor starves the DVE
# (measured 3.3x slowdown on overlapping DVE ops), so everything elementwise
# stays on DVE/ACT and sums go to the PE.
PE_REDUCE = {(0, 0), (1, 1), (2, 2), (1, 2)}  # PE chunk-reduced terms
DX_PE = (True, True, False)  # dx channel sums on PE (else DVE adds)
DR_PE = (False, True, True)  # dr channel assembly on PE (else DVE TS+sub)


def _build_program(ca: float, cid: float, n_reps: int = 1):
    """Build + compile the per-core SPMD bass program.

    ca  = alpha / (B * normalization)   (scale for the raw sum S_ij)
    cid = (1 - alpha) / M_INV           (identity mixture term)
    """
    from contextlib import ExitStack

    import concourse.bacc as bacc
    import concourse.mybir as mybir
    import concourse.tile as tile

    dt = mybir.dt
    op = mybir.AluOpType
    f32 = dt.float32
    f16 = dt.float16
    AF = mybir.ActivationFunctionType

    nc = bacc.Bacc("TRN2", target_bir_lowering=False, debug=False,
                   num_devices=N_CORES)

    s_in = nc.dram_tensor("s_in", [C, P, B, PL], f16,
                          kind="ExternalInput").ap()
    u_in = nc.dram_tensor("u_in", [2 * C, P, B, PL], f16,
                          kind="ExternalInput").ap()
    id_in = nc.dram_tensor("ident", [P, P], dt.float16,
                           kind="ExternalInput").ap()
    drift_o = nc.dram_tensor("drift", [2 * C, P, B, PL], f16,
                             kind="ExternalOutput").ap()
    dif_o = nc.dram_tensor("dif", [C, P, PL], f32, kind="ExternalOutput").ap()

    HB = B // 2   # batch half
    QB = B // 4   # batch quarter

    with tile.TileContext(nc) as tc:
      for _rep in range(n_reps):
        with ExitStack() as stack:
            coef = stack.enter_context(tc.tile_pool(name="coef", bufs=1))
            data = stack.enter_context(tc.tile_pool(name="data", bufs=1))
            tmp = stack.enter_context(tc.tile_pool(name="tmp", bufs=2))
            ident = coef.tile([P, P], f16, tag="ident")
            bgid = coef.tile([P, P], f16, tag="bgid")

            g = {}
            with tc.tile_pool(name="score", bufs=1) as score_pool, \
                 tc.tile_pool(name="prod", bufs=1) as prod_pool, \
                 tc.tile_pool(name="psA", bufs=1, space="PSUM") as psA:
                # ---------------- input DMA ----------------
                s_t = [score_pool.tile([P, B, PL], f16, tag=f"s{c}",
                                       name=f"s{c}") for c in range(C)]
                for bh in range(2):
                    bsl = slice(bh * HB, (bh + 1) * HB)
                    for c in range(C):
                        nc.sync.dma_start(out=s_t[c][:, bsl, :],
                                          in_=s_in[c, :, bsl, :])
                nc.sync.dma_start(out=ident[:], in_=id_in[:])
                nc.scalar.mul(bgid[:], ident[:], BG)

                # ---------------- stage A ----------------
                # products: squares on ACT, crosses on DVE/GP (b-halved so
                # work starts as soon as the first half-DMAs land)
                q = {}
                for i in range(C):
                    sq = prod_pool.tile([P, B, PL], f16, tag=f"sq{i}")
                    for bh in range(2):
                        bsl = slice(bh * HB, (bh + 1) * HB)
                        nc.scalar.activation(sq[:, bsl, :], s_t[i][:, bsl, :],
                                             AF.Square)
                    q[(i, i)] = sq
                for (i, j) in [(0, 1), (0, 2), (1, 2)]:
                    qt = prod_pool.tile([P, B, PL], f16, tag=f"q{i}{j}")
                    for bh in range(2):
                        bsl = slice(bh * HB, (bh + 1) * HB)
                        nc.vector.tensor_tensor(qt[:, bsl, :],
                                                s_t[i][:, bsl, :],
                                                s_t[j][:, bsl, :], op.mult)
                    q[(i, j)] = qt

                # u lands while stage A runs: x first (dr path), then r
                u_t = []
                for c in range(2 * C):
                    ut = data.tile([P, B, PL], f16, tag=f"u{c}")
                    for bh in range(2):
                        bsl = slice(bh * HB, (bh + 1) * HB)
                        nc.sync.dma_start(out=ut[:, bsl, :],
                                          in_=u_in[c, :, bsl, :])
                    u_t.append(ut)
                x_t, r_t = u_t[:C], u_t[C:]

                # reductions over batch -> S planes [P, PL] f32-ish
                S = {}
                for key in [(0, 0), (1, 1), (2, 2), (0, 1), (0, 2), (1, 2)]:
                    qt = q[key]
                    if key in PE_REDUCE:
                        ps = psA.tile([P, 8 * PL], f32, tag=f"psA{key[0]}{key[1]}")
                        for ck in range(8):
                            rhs = qt[:, ck * 8:(ck + 1) * 8, :].rearrange(
                                "p b l -> p (b l)")
                            nc.tensor.matmul(ps[:], ident[:], rhs,
                                             start=(ck == 0), stop=(ck == 7))
                        r8 = tmp.tile([P, 8, PL], f32, tag="r8")
                        nc.scalar.copy(
                            r8[:], ps[:].rearrange("p (b l) -> p b l", b=8))
                        r4 = tmp.tile([P, 4, PL], f32, tag="r4")
                        nc.vector.tensor_tensor(r4[:], r8[:, 0:4, :],
                                                r8[:, 4:8, :], op.add)
                        r2 = tmp.tile([P, 2, PL], f32, tag="r2")
                        nc.vector.tensor_tensor(r2[:], r4[:, 0:2, :],
                                                r4[:, 2:4, :], op.add)
                        st = tmp.tile([P, PL], f32, tag=f"S{key[0]}{key[1]}")
                        nc.vector.tensor_tensor(st[:], r2[:, 0, :],
                                                r2[:, 1, :], op.add)
                        S[key] = st
                    else:
                        # fp16 fold chain 64 -> 2 on DVE, final add emits f32
                        cur = qt
                        w = B
                        while w > 2:
                            h = w // 2
                            nxt = tmp.tile([P, h, PL], f16, tag=f"f{h}",
                                           bufs=3)
                            nc.vector.tensor_tensor(
                                nxt[:], cur[:, 0:h, :], cur[:, h:w, :],
                                op.add)
                            cur = nxt
                            w = h
                        st = tmp.tile([P, PL], f32, tag=f"S{key[0]}{key[1]}")
                        nc.vector.tensor_tensor(st[:], cur[:, 0, :],
                                                cur[:, 1, :], op.add)
                        S[key] = st

                for i in range(C):
                    gii = coef.tile([P, PL], f32, tag=f"g{i}{i}")
                    nc.scalar.activation(gii[:], S[(i, i)][:], AF.Copy,
                                         bias=float(cid), scale=float(ca))
                    g[(i, i)] = gii
                for (i, j) in [(0, 1), (0, 2), (1, 2)]:
                    gij = coef.tile([P, PL], f32, tag=f"g{i}{j}")
                    nc.scalar.mul(gij[:], S[(i, j)][:], float(ca))
                    g[(i, j)] = gij
                    g[(j, i)] = gij

            # ------------- stage B: per-pixel 3x3 coefficients -------------
            def tt(a, b_, o, tag):
                t = coef.tile([P, PL], f32, tag=tag)
                nc.vector.tensor_tensor(t[:], a[:], b_[:], o)
                return t

            def fmsub(a, b_, c_, d_, tag):
                # a*b - c*d
                t1 = tmp.tile([P, PL], f32, tag="fm1")
                nc.vector.tensor_tensor(t1[:], a[:], b_[:], op.mult)
                t2 = tmp.tile([P, PL], f32, tag="fm2")
                nc.vector.tensor_tensor(t2[:], c_[:], d_[:], op.mult)
                t = coef.tile([P, PL], f32, tag=tag)
                nc.vector.tensor_tensor(t[:], t1[:], t2[:], op.subtract)
                return t

            def to16(plane, tag):
                e = coef.tile([P, 1, PL], f16, tag=tag)
                nc.scalar.copy(e[:, 0, :], plane[:])
                return e

            def sqrt_p(a, tag):
                out = coef.tile([P, PL], f32, tag=tag)
                nc.scalar.activation(out[:], a[:], AF.Sqrt)
                return out

            l00 = sqrt_p(g[0, 0], "l00")
            il00 = coef.tile([P, PL], f32, tag="il00")
            nc.vector.reciprocal(il00[:], l00[:])
            l10 = tt(g[0, 1], il00, op.mult, "l10")
            l20 = tt(g[0, 2], il00, op.mult, "l20")
            t = tt(l10, l10, op.mult, "l10sq")
            dd1 = tt(g[1, 1], t, op.subtract, "dd1")
            l11 = sqrt_p(dd1, "l11")
            il11 = coef.tile([P, PL], f32, tag="il11")
            nc.vector.reciprocal(il11[:], l11[:])
            t = tt(l20, l10, op.mult, "l20l10")
            t = tt(g[1, 2], t, op.subtract, "g12m")
            l21 = tt(t, il11, op.mult, "l21")
            t = tt(l20, l20, op.mult, "l20sq")
            dd2 = tt(g[2, 2], t, op.subtract, "dd2a")
            t = tt(l21, l21, op.mult, "l21sq")
            dd2 = tt(dd2, t, op.subtract, "dd2")
            l22 = sqrt_p(dd2, "l22")

            # bL = beta * L  (scaled once, reused by drift_r, A, diffusion)
            L = {}
            for (i, j), lt in [((0, 0), l00), ((1, 0), l10), ((1, 1), l11),
                               ((2, 0), l20), ((2, 1), l21), ((2, 2), l22)]:
                blt = coef.tile([P, PL], f32, tag=f"bl{i}{j}")
                nc.scalar.mul(blt[:], lt[:], BETA_C)
                L[(i, j)] = blt
            eL = {(i, j): to16(L[(i, j)], f"eL{i}{j}")[:]
                  for (i, j) in [(0, 0), (1, 0), (1, 1),
                                 (2, 0), (2, 1), (2, 2)]}

            # diffusion_r rows (batch-independent): bg/beta * row sums of bL
            bgob = BG_SCALE / BETA_C
            dif0 = coef.tile([P, PL], f32, tag="dif0")
            nc.scalar.mul(dif0[:], L[0, 0][:], bgob)
            t = tt(L[1, 0], L[1, 1], op.add, "difs1")
            dif1 = coef.tile([P, PL], f32, tag="dif1")
            nc.scalar.mul(dif1[:], t[:], bgob)
            t = tt(L[2, 0], L[2, 1], op.add, "difs2a")
            t = tt(t, L[2, 2], op.add, "difs2")
            dif2 = coef.tile([P, PL], f32, tag="dif2")
            nc.scalar.mul(dif2[:], t[:], bgob)
            for i, dtile in enumerate((dif0, dif1, dif2)):
                nc.sync.dma_start(out=dif_o[i], in_=dtile[:])

            # Ginv = W^T W with W = L^{-1} (lower).  Unsigned variants
            # w10' = -w10, w21' = -w21 keep the chain negation-free; the two
            # sign flips land on the ACT copies into the stacked IV rows.
            il22 = coef.tile([P, PL], f32, tag="il22")
            nc.vector.reciprocal(il22[:], l22[:])
            t_a = tt(l10, il00, op.mult, "ta")        # l10/l00
            w10p = tt(t_a, il11, op.mult, "w10p")     # -w10
            t_b = tt(l21, il11, op.mult, "tb")
            w21p = tt(t_b, il22, op.mult, "w21p")     # -w21
            t1 = tt(l10, l21, op.mult, "wt1")
            t2 = tt(l20, l11, op.mult, "wt2")
            t3 = tt(t1, t2, op.subtract, "wt3")
            t4 = tt(il00, il11, op.mult, "wt4")
            t5 = tt(t3, t4, op.mult, "wt5")
            w20 = tt(t5, il22, op.mult, "w20")        # +w20 (signed)

            # Ginv rows stacked as [P, 3(j), PL]
            IV = [coef.tile([P, 3, PL], f32, tag=f"IV{k}", name=f"IV{k}")
                  for k in range(3)]
            # iv00 = w00^2 + w10'^2 + w20^2
            s0 = tt(il00, il00, op.mult, "iv_s0")
            s1 = tt(w10p, w10p, op.mult, "iv_s1")
            s2_ = tt(w20, w20, op.mult, "iv_s2")
            s01_ = tt(s0, s1, op.add, "iv_s01")
            nc.vector.tensor_tensor(IV[0][:, 0, :], s01_[:], s2_[:], op.add)
            # iv01 = -(w10'*w11 + w20*w21')
            n0 = tt(w10p, il11, op.mult, "iv_n0")
            n1 = tt(w20, w21p, op.mult, "iv_n1")
            n01 = tt(n0, n1, op.add, "iv_n01")
            nc.scalar.mul(IV[0][:, 1, :], n01[:], -1.0)
            nc.scalar.mul(IV[1][:, 0, :], n01[:], -1.0)
            # iv02 = w20*w22
            nc.vector.tensor_tensor(IV[0][:, 2, :], w20[:], il22[:], op.mult)
            nc.scalar.copy(IV[2][:, 0, :], IV[0][:, 2, :])
            # iv11 = w11^2 + w21'^2
            m0 = tt(il11, il11, op.mult, "iv_m0")
            m1 = tt(w21p, w21p, op.mult, "iv_m1")
            nc.vector.tensor_tensor(IV[1][:, 1, :], m0[:], m1[:], op.add)
            # iv12 = -(w21'*w22)
            m2 = tt(w21p, il22, op.mult, "iv_m2")
            nc.scalar.mul(IV[1][:, 2, :], m2[:], -1.0)
            nc.scalar.mul(IV[2][:, 1, :], m2[:], -1.0)
            # iv22 = w22^2
            nc.vector.tensor_tensor(IV[2][:, 2, :], il22[:], il22[:], op.mult)

            # A-row i = sum_{k<=i} bL_ik (broadcast over j) * IV_k
            def blb(i, k):
                return L[(i, k)][:].rearrange(
                    "p l -> p () l").broadcast_to([P, 3, PL])

            AR = []
            for i in range(3):
                ar = coef.tile([P, 3, PL], f32, tag=f"AR{i}", name=f"AR{i}")
                if i == 0:
                    nc.vector.tensor_tensor(ar[:], IV[0][:], blb(0, 0),
                                            op.mult)
                else:
                    acc = tmp.tile([P, 3, PL], f32, tag="Aacc")
                    nc.vector.tensor_tensor(acc[:], IV[0][:], blb(i, 0),
                                            op.mult)
                    for k in range(1, i + 1):
                        pr = tmp.tile([P, 3, PL], f32, tag="Apr")
                        nc.vector.tensor_tensor(pr[:], IV[k][:], blb(i, k),
                                                op.mult)
                        dst = ar if k == i else tmp.tile([P, 3, PL], f32,
                                                         tag="Aacc")
                        nc.vector.tensor_tensor(dst[:], acc[:], pr[:], op.add)
                        acc = dst
                AR.append(ar)
            eAR = []
            for i in range(3):
                e = coef.tile([P, 3, PL], f16, tag=f"eAR{i}", name=f"eAR{i}")
                nc.scalar.copy(e[:], AR[i][:])
                eAR.append(e)
            eA = {(i, j): eAR[i][:, j:j + 1, :]
                  for i in range(3) for j in range(3)}

            mtmp = stack.enter_context(tc.tile_pool(name="mtmp", bufs=2))
            outs = stack.enter_context(tc.tile_pool(name="outs", bufs=1))
            psum = stack.enter_context(
                tc.tile_pool(name="psum", bufs=2, space="PSUM"))

            # ------------- stage C: batched main stage ---------------------
            # full-batch products on DVE; channel sums accumulated on PE via
            # identity matmuls into PSUM (term-major so weights stay loaded);
            # the dr BG*r term rides the same accumulation through a
            # BG-scaled identity; ACT drains with the channel sign and the
            # output DMA streams per batch-quarter.
            def products(coeffs, ins):
                prs = []
                for idx, (cc, dd) in enumerate(zip(coeffs, ins)):
                    pr = mtmp.tile([P, B, PL], f16, tag=f"pr{idx}", bufs=2)
                    bc = cc.broadcast_to([P, B, PL])
                    nc.vector.tensor_tensor(pr[:], dd[:], bc, op.mult)
                    prs.append(pr)
                return prs

            def pe_channel(tag, prs, extra_rhs, scale, dma_out):
                # rhs list: (flat AP, weights) pairs
                rhss = [(pr[:].rearrange("p b l -> p (b l)"), ident[:])
                        for pr in prs]
                if extra_rhs is not None:
                    rhss.append((extra_rhs[:].rearrange("p b l -> p (b l)"),
                                 bgid[:]))
                n = len(rhss)
                for bq in range(4):
                    ps = psum.tile([P, 1024], f32, tag="ps", bufs=4)
                    for idx, (rhs, wt) in enumerate(rhss):
                        for s2 in range(2):
                            sl = slice(s2 * 512, (s2 + 1) * 512)
                            gl = slice(bq * 1024 + s2 * 512,
                                       bq * 1024 + (s2 + 1) * 512)
                            nc.tensor.matmul(
                                ps[:, sl], wt, rhs[:, gl],
                                start=(idx == 0), stop=(idx == n - 1))
                    qsl = slice(bq * 16, (bq + 1) * 16)
                    qo = outs.tile([P, 16, PL], f16, tag="qo", bufs=4)
                    nc.scalar.mul(
                        qo[:], ps[:].rearrange("p (b l) -> p b l", b=16),
                        scale)
                    nc.sync.dma_start(out=dma_out[:, qsl, :], in_=qo[:])

            def emit_dx(i):
                coeffs = [eA[(i, 0)], eA[(i, 1)], eA[(i, 2)]]
                prs = products(coeffs, r_t)
                if DX_PE[i]:
                    pe_channel(f"dx{i}", prs, None, 1.0, drift_o[i])
                else:
                    dx = outs.tile([P, B, PL], f16, tag="full", bufs=2)
                    s01 = mtmp.tile([P, B, PL], f16, tag="s01", bufs=1)
                    nc.vector.tensor_tensor(s01[:], prs[0][:], prs[1][:],
                                            op.add)
                    nc.vector.tensor_tensor(dx[:], s01[:], prs[2][:], op.add)
                    nc.sync.dma_start(out=drift_o[i], in_=dx[:])

            def emit_dr(i):
                coeffs = [eL[(i, j)] for j in range(i + 1)]
                prs = products(coeffs, x_t)
                if DR_PE[i]:
                    pe_channel(f"dr{i}", prs, r_t[i], -1.0, drift_o[C + i])
                else:
                    dr = outs.tile([P, B, PL], f16, tag="full", bufs=2)
                    acc = prs[0]
                    for k in range(1, len(prs)):
                        nxt = mtmp.tile([P, B, PL], f16, tag="racc", bufs=1)
                        nc.vector.tensor_tensor(nxt[:], acc[:], prs[k][:],
                                                op.add)
                        acc = nxt
                    negr = mtmp.tile([P, B, PL], f16, tag="negr", bufs=1)
                    nc.vector.tensor_scalar(negr[:], r_t[i][:], -BG,
                                            None, op.mult)
                    nc.vector.tensor_tensor(dr[:], negr[:], acc[:],
                                            op.subtract)
                    nc.sync.dma_start(out=drift_o[C + i], in_=dr[:])

            # dr first (needs only L), dx after (needs A); dr0/dx2 are
            # DVE-assembled and go last so the PE/ACT pipelines stay fed.
            emit_dr(2)
            emit_dr(1)
            emit_dr(0)
            emit_dx(0)
            emit_dx(1)
            emit_dx(2)

    nc.compile()
    return nc


def _host_reference(u, score_x, t):
    """Pure-numpy fallback (exact reference math); used only when t[0]==1.0
    (the stateful normalization branch, never hit with uniform t)."""
    x, r = u[:, :C], u[:, C:]
    s = np.transpose(score_x, (0, 2, 3, 1)).astype(np.float32)
    G = np.einsum("bhwi,bhwj->hwij", s, s) / np.float32(score_x.shape[0])
    t0 = t[0]
    diag_mean = np.mean(np.trace(G, axis1=-2, axis2=-1)) / C
    normalization = np.where(t0 == 1.0, diag_mean * M_INV, 1.0)
    G = G / normalization
    G_id = (1.0 / M_INV) * np.eye(C, dtype=G.dtype)
    alpha = RIEMANN_MIX * np.exp(-K_DECAY * (1.0 - t0))
    G = alpha * G + (1.0 - alpha) * G_id
    G_inv = np.linalg.inv(G).astype(np.float32)
    G_sqrt = np.linalg.cholesky(G).astype(np.float32)

    def mm(Amat, Bf):
        return np.einsum("hwij,bjhw->bihw", Amat, Bf).astype(np.float32)

    hr = mm(G_inv, r)
    drift_x = BETA_C * mm(G_sqrt, hr)
    drift_r = (-BETA_C * mm(G_sqrt, x) - BETA_C * GAMMA_C * mm(G, hr))
    diffusion_x = np.zeros_like(x)
    diffusion_r = BG_SCALE * mm(G_sqrt, np.ones_like(r))
    drift = np.concatenate((drift_x, drift_r), axis=1)
    diffusion = np.concatenate((diffusion_x, diffusion_r), axis=1)
    return drift, diffusion


def kernel(u, score_x, t):
    from concourse.bass_utils import run_bass_kernel_spmd

    u = np.ascontiguousarray(np.asarray(u, dtype=np.float32))
    score_x = np.ascontiguousarray(np.asarray(score_x, dtype=np.float32))
    t = np.asarray(t, dtype=np.float32)

    t0 = float(t[0])
    if t0 == 1.0:
        return _host_reference(u, score_x, t)

    alpha = RIEMANN_MIX * math.exp(-K_DECAY * (1.0 - t0))
    ca = alpha / B          # normalization == 1.0 on this branch
    cid = (1.0 - alpha) / M_INV

    key = (round(ca, 12), round(cid, 12))
    nc = _PROG_CACHE.get(key)
    if nc is None:
        nc = _build_program(ca, cid)
        _PROG_CACHE[key] = nc

    in_maps = []
    for k in range(N_CORES):
        rows = slice(k * ROWS, (k + 1) * ROWS)
        s_np = (score_x[:, :, rows, :]
                .reshape(B, C, P, PL).transpose(1, 2, 0, 3)
                .astype(np.float16))
        u_np = (u[:, :, rows, :]
                .reshape(B, 2 * C, P, PL).transpose(1, 2, 0, 3)
                .astype(np.float16))
        in_maps.append({
            "s_in": np.ascontiguousarray(s_np),
            "u_in": np.ascontiguousarray(u_np),
            "ident": np.eye(P, dtype=np.float16),
        })

    res = run_bass_kernel_spmd(nc, in_maps, list(range(N_CORES)))

    drift = np.empty((B, 2 * C, HW, HW), dtype=np.float32)
    diffusion = np.zeros((B, 2 * C, HW, HW), dtype=np.float32)
    for k in range(N_CORES):
        rows = slice(k * ROWS, (k + 1) * ROWS)
        dk = res.results[k]["drift"].astype(np.float32)     # [6, P, B, PL]
        drift[:, :, rows, :] = dk.transpose(2, 0, 1, 3).reshape(
            B, 2 * C, ROWS, HW)
        difk = res.results[k]["dif"].reshape(C, ROWS, HW)   # [3, P, PL]
        diffusion[:, C:, rows, :] = difk[None, :, :, :]
    return drift, diffusion
